# revision 3
# baseline (speedup 1.0000x reference)
"""TRN2 Bass kernel for nn_AttentionMechanism (visual-text attention).

  v = visual @ Wv.T + bv          (128, 256, 1024)
  t = text @ Wt.T + bt            (128, 1024)
  combined = tanh(v + t[:, None])
  scores = combined @ Wa[0] + ba  (128, 256)   [ba dropped: softmax shift-inv]
  attention_weights = softmax(scores, axis=1)
  attended = einsum('br,brd->bd', attention_weights, visual)

Data-parallel across batch on 8 NeuronCores (16 batches/core). Per core the
big projection runs on the TensorEngine in float32r (tf32-like, ~1 cyc/row at
N>=256, ~13-bit mantissa) with the hidden dim on PSUM partitions (vT
orientation) so the per-batch text bias folds into the ScalarEngine's
per-partition activation bias and the score reduction is a PE matvec. The
attended weighted-sum also runs on the PE against a second, natural-layout
copy of visual. Softmax + attended for rowgroup g are deferred until after
rowgroup g+1's matmuls are emitted so the PE never stalls on them.
"""

import sys

if "/opt/trn_rl_repo" not in sys.path:
    sys.path.insert(0, "/opt/trn_rl_repo")

import os
from contextlib import ExitStack

import numpy as np

# ---- problem constants (hardcoded per contract) ----
B, R, VD, TD, H = 128, 256, 2048, 1024, 1024
NCORES = 8
BC = B // NCORES          # 16 batches per core
RGB = 2                   # batches per rowgroup
NG = BC // RGB            # 8 rowgroups
RW = RGB * R              # 512 rows per rowgroup
KV = VD // 128            # 16 k-tiles (visual dim)
KT = TD // 128            # 8 k-tiles (text dim)
NH = H // 128             # 8 h-tiles
ND = VD // 512            # 4 n-tiles for attended output
RK = R // 128             # 2 r-tiles per batch

_cache = {}


def _install_ntff_hook():
    """Register the axon NTFF profile hook the agent image's antenv lacks."""
    import types

    import antenv

    if "antenv.axon_hooks" not in sys.modules:
        mod = types.ModuleType("antenv.axon_hooks")
        mod._hook = None
        mod.set_axon_ntff_profile_hook = lambda h: setattr(mod, "_hook", h)
        mod.get_axon_ntff_profile_hook = lambda: mod._hook
        sys.modules["antenv.axon_hooks"] = mod
        antenv.axon_hooks = mod
    sys.path.insert(0, "/root/.axon_site/trn_agent_boot")
    import trn_boot
    hook = trn_boot._ntff_profile_via_ctypes("/opt/axon/libaxon_pjrt.so")
    sys.modules["antenv.axon_hooks"].set_axon_ntff_profile_hook(hook)


def _build_nc():
    import concourse.bass as bass
    import concourse.tile as tile
    from concourse import bacc, mybir

    F32 = mybir.dt.float32
    F32R = mybir.dt.float32r
    AF = mybir.ActivationFunctionType

    nc = bacc.Bacc("TRN2", target_bir_lowering=False, debug=False,
                   num_devices=NCORES)

    visT_d = nc.dram_tensor("visT", [NG, VD, RW], F32, kind="ExternalInput")
    nat_d = nc.dram_tensor("visnat", [BC, R, VD], F32, kind="ExternalInput")
    WvT_d = nc.dram_tensor("WvT", [VD, H], F32, kind="ExternalInput")
    WtT_d = nc.dram_tensor("WtT", [TD, H], F32, kind="ExternalInput")
    textT_d = nc.dram_tensor("textT", [TD, BC], F32, kind="ExternalInput")
    bias_d = nc.dram_tensor("biasvb", [128, NH], F32, kind="ExternalInput")
    waT_d = nc.dram_tensor("waT", [128, NH], F32, kind="ExternalInput")
    att_d = nc.dram_tensor("attended", [BC, VD], F32, kind="ExternalOutput")
    aw_d = nc.dram_tensor("attnw", [BC, R], F32, kind="ExternalOutput")

    with tile.TileContext(nc) as tc, ExitStack() as ctx:
        consts = ctx.enter_context(tc.tile_pool(name="consts", bufs=1))
        wtp = ctx.enter_context(tc.tile_pool(name="wtp", bufs=3))
        visp = ctx.enter_context(tc.tile_pool(name="visp", bufs=24))
        natp = ctx.enter_context(tc.tile_pool(name="natp", bufs=2))
        combp = ctx.enter_context(tc.tile_pool(name="combp", bufs=2))
        smallp = ctx.enter_context(tc.tile_pool(name="smallp", bufs=3))
        dramp = ctx.enter_context(tc.tile_pool(name="dramp", bufs=3, space="DRAM"))
        vtps = ctx.enter_context(tc.tile_pool(name="vtps", bufs=3, space="PSUM"))
        scps = ctx.enter_context(tc.tile_pool(name="scps", bufs=2, space="PSUM"))
        atps = ctx.enter_context(tc.tile_pool(name="atps", bufs=2, space="PSUM"))
        tps = ctx.enter_context(tc.tile_pool(name="tps", bufs=1, space="PSUM"))

        # ---- resident constants ----
        WvT_sb = consts.tile([128, KV, NH, 128], F32R, tag="WvT")
        wv_re = WvT_d[:, :].rearrange("(k p) (h m) -> p k h m", p=128, m=128)
        for k in range(KV):
            nc.sync.dma_start(WvT_sb[:, k, :, :], wv_re[:, k, :, :].bitcast(F32R))
        textT_sb = consts.tile([128, KT, BC], F32R, tag="textT")
        nc.sync.dma_start(
            textT_sb[:],
            textT_d[:, :].rearrange("(k p) b -> p k b", p=128).bitcast(F32R))
        bias_sb = consts.tile([128, NH], F32, tag="bias")
        nc.sync.dma_start(bias_sb[:], bias_d[:, :])
        waT_sb = consts.tile([128, NH], F32R, tag="waT")
        nc.sync.dma_start(waT_sb[:], waT_d[:, :].bitcast(F32R))
        tT_sb = consts.tile([128, NH, BC], F32, tag="tT")

        # ---- phase 0: tT[h, b] = (WtT.T @ textT)[h, b] + (bv + bt)[h] ----
        wt_re = WtT_d[:, :].rearrange("(k p) (h m) -> p k h m", p=128, m=128)
        for h in range(NH):
            t_ps = tps.tile([128, BC], F32, tag="tps")
            for k in range(KT):
                wt = wtp.tile([128, 128], F32R, tag="wt")
                nc.sync.dma_start(wt[:], wt_re[:, k, h, :].bitcast(F32R))
                nc.tensor.matmul(t_ps[:], wt[:], textT_sb[:, k, :],
                                 start=(k == 0), stop=(k == KT - 1))
            nc.vector.tensor_scalar_add(tT_sb[:, h, :], t_ps[:],
                                        bias_sb[:, h:h + 1])

        # ---- phase 1: main loop over rowgroups, tail deferred by one ----
        def emit_head(g):
            vis_k = []
            for k in range(KV):
                vk = visp.tile([128, RW], F32R, tag="vis")
                nc.sync.dma_start(
                    vk[:], visT_d[g, k * 128:(k + 1) * 128, :].bitcast(F32R))
                vis_k.append(vk)
            comb = combp.tile([128, NH, RW], F32R, tag="comb")
            sc_ps = scps.tile([1, RW], F32, tag="sc")
            for h in range(NH):
                vt = vtps.tile([128, RW], F32, tag="vt")
                for k in range(KV):
                    nc.tensor.matmul(vt[:], WvT_sb[:, k, h, :], vis_k[k][:],
                                     start=(k == 0), stop=(k == KV - 1))
                for j in range(RGB):
                    b = g * RGB + j
                    nc.scalar.activation(
                        comb[:, h, j * R:(j + 1) * R], vt[:, j * R:(j + 1) * R],
                        AF.Tanh, bias=tT_sb[:, h, b:b + 1], scale=1.0)
                nc.tensor.matmul(sc_ps[:], waT_sb[:, h:h + 1], comb[:, h, :],
                                 start=(h == 0), stop=(h == NH - 1))
            return sc_ps

        def emit_tail(g, sc_ps):
            for j in range(RGB):
                b = g * RGB + j
                sl = slice(j * R, (j + 1) * R)
                nmax = smallp.tile([1, 1], F32, tag="nmax")
                nc.vector.reduce_max(nmax[:], sc_ps[:, sl],
                                     axis=mybir.AxisListType.X, negate=True)
                ex = smallp.tile([1, R], F32, tag="ex")
                esum = smallp.tile([1, 1], F32, tag="esum")
                nc.scalar.activation(ex[:], sc_ps[:, sl], AF.Exp,
                                     bias=nmax[:], scale=1.0, accum_out=esum[:])
                rec = smallp.tile([1, 1], F32, tag="rec")
                nc.vector.reciprocal(rec[:], esum[:])
                wsm = smallp.tile([1, R], F32, tag="wsm")
                nc.vector.tensor_scalar_mul(wsm[:], ex[:], rec[:])
                nc.gpsimd.dma_start(aw_d[b:b + 1, :], wsm[:])
                # bounce through DRAM to get w onto partitions: [1,R]->[128,RK]
                wb = dramp.tile([1, R], F32, tag="wb")
                nc.gpsimd.dma_start(wb[:], wsm[:])
                wcol = smallp.tile([128, RK], F32R, tag="wcol")
                src = wb[:]
                scat = bass.AP(tensor=src.tensor, offset=src.offset,
                               ap=[[1, 128], [128, RK]])
                nc.gpsimd.dma_start(wcol[:], scat.bitcast(F32R))
                nat = natp.tile([128, RK, VD], F32R, tag="nat")
                nc.scalar.dma_start(
                    nat[:],
                    nat_d[b, :, :].rearrange("(rk p) d -> p rk d", p=128).bitcast(F32R))
                for n in range(ND):
                    att_ps = atps.tile([1, 512], F32, tag="att")
                    for rk in range(RK):
                        nc.tensor.matmul(att_ps[:], wcol[:, rk:rk + 1],
                                         nat[:, rk, n * 512:(n + 1) * 512],
                                         start=(rk == 0), stop=(rk == RK - 1))
                    att_sb = smallp.tile([1, 512], F32, tag="attsb")
                    nc.vector.tensor_copy(att_sb[:], att_ps[:])
                    nc.gpsimd.dma_start(att_d[b:b + 1, n * 512:(n + 1) * 512],
                                        att_sb[:])

        prev = None
        for g in range(NG):
            state = emit_head(g)
            if prev is not None:
                emit_tail(g - 1, prev)
            prev = state
        emit_tail(NG - 1, prev)

    nc.compile()
    return nc


def _prep_inputs(visual_features, text_features, Wv, bv, Wt, bt, Wa, ba):
    f32 = np.float32
    vis = np.ascontiguousarray(visual_features, dtype=f32)
    txt = np.ascontiguousarray(text_features, dtype=f32)
    WvT = np.ascontiguousarray(np.asarray(Wv, f32).T)                # (VD, H)
    WtT = np.ascontiguousarray(np.asarray(Wt, f32).T)                # (TD, H)
    bias_vb = np.asarray(bv, f32) + np.asarray(bt, f32)              # (H,)
    bias_sb = np.ascontiguousarray(bias_vb.reshape(NH, 128).T)       # (128, NH)
    waT = np.ascontiguousarray(np.asarray(Wa, f32)[0].reshape(NH, 128).T)

    in_maps = []
    for c in range(NCORES):
        vb = vis[c * BC:(c + 1) * BC]                                # (BC, R, VD)
        vT = vb.transpose(0, 2, 1)                                   # (BC, VD, R)
        visT = np.ascontiguousarray(
            vT.reshape(NG, RGB, VD, R).transpose(0, 2, 1, 3).reshape(NG, VD, RW))
        textT = np.ascontiguousarray(txt[c * BC:(c + 1) * BC].T)     # (TD, BC)
        in_maps.append({
            "visT": visT,
            "visnat": np.ascontiguousarray(vb),
            "WvT": WvT,
            "WtT": WtT,
            "textT": textT,
            "biasvb": bias_sb,
            "waT": waT,
        })
    return in_maps


def kernel(visual_features, text_features, Wv, bv, Wt, bt, Wa, ba):
    from concourse.bass_utils import run_bass_kernel_spmd

    trace = bool(os.environ.get("KERNEL_TRACE"))
    if trace:
        try:
            _install_ntff_hook()
        except Exception as e:  # profiling is best-effort
            print("ntff hook install failed:", e)
            trace = False

    if "nc" not in _cache:
        _cache["nc"] = _build_nc()
    nc = _cache["nc"]

    in_maps = _prep_inputs(visual_features, text_features, Wv, bv, Wt, bt,
                           Wa, ba)
    res = run_bass_kernel_spmd(nc, in_maps, list(range(NCORES)), trace=trace)
    if trace and res.exec_time_ns is not None:
        print(f"HW exec time: {res.exec_time_ns} ns")

    attended = np.concatenate([r["attended"] for r in res.results], axis=0)
    attn_w = np.concatenate([r["attnw"] for r in res.results], axis=0)
    return attended.astype(np.float32), attn_w.astype(np.float32)


# revision 8
# speedup vs baseline: 1.1341x; 1.1341x over previous
"""TRN2 Bass kernel for nn_AttentionMechanism (visual-text attention).

  v = visual @ Wv.T + bv          (128, 256, 1024)
  t = text @ Wt.T + bt            (128, 1024)
  combined = tanh(v + t[:, None])
  scores = combined @ Wa[0] + ba  (128, 256)   [ba dropped: softmax shift-inv]
  attention_weights = softmax(scores, axis=1)
  attended = einsum('br,brd->bd', attention_weights, visual)

Data-parallel across batch on 8 NeuronCores (16 batches/core). Per core the
big projection runs on the TensorEngine in float32r (tf32-like, ~1 cyc/row at
N>=256, ~13-bit mantissa) with the hidden dim on PSUM partitions (vT
orientation) so the per-batch text bias folds into the ScalarEngine's
per-partition activation bias and the score reduction is a PE matvec. The
attended weighted-sum also runs on the PE against a second, natural-layout
copy of visual. Softmax + attended for rowgroup g are deferred until after
rowgroup g+1's matmuls are emitted so the PE never stalls on them.
"""

import sys

if "/opt/trn_rl_repo" not in sys.path:
    sys.path.insert(0, "/opt/trn_rl_repo")

import os
from contextlib import ExitStack

import numpy as np

# ---- problem constants (hardcoded per contract) ----
B, R, VD, TD, H = 128, 256, 2048, 1024, 1024
NCORES = 8
BC = B // NCORES          # 16 batches per core
RGB = 2                   # batches per rowgroup
NG = BC // RGB            # 8 rowgroups
RW = RGB * R              # 512 rows per rowgroup
KV = VD // 128            # 16 k-tiles (visual dim)
KT = TD // 128            # 8 k-tiles (text dim)
NH = H // 128             # 8 h-tiles
ND = VD // 512            # 4 n-tiles for attended output
RK = R // 128             # 2 r-tiles per batch

_cache = {}


def _install_ntff_hook():
    """Register the axon NTFF profile hook the agent image's antenv lacks."""
    import types

    import antenv

    if "antenv.axon_hooks" not in sys.modules:
        mod = types.ModuleType("antenv.axon_hooks")
        mod._hook = None
        mod.set_axon_ntff_profile_hook = lambda h: setattr(mod, "_hook", h)
        mod.get_axon_ntff_profile_hook = lambda: mod._hook
        sys.modules["antenv.axon_hooks"] = mod
        antenv.axon_hooks = mod
    sys.path.insert(0, "/root/.axon_site/trn_agent_boot")
    import trn_boot
    hook = trn_boot._ntff_profile_via_ctypes("/opt/axon/libaxon_pjrt.so")
    sys.modules["antenv.axon_hooks"].set_axon_ntff_profile_hook(hook)


def _build_nc():
    import concourse.bass as bass
    import concourse.tile as tile
    from concourse import bacc, mybir

    F32 = mybir.dt.float32
    F32R = mybir.dt.float32r
    AF = mybir.ActivationFunctionType

    nc = bacc.Bacc("TRN2", target_bir_lowering=False, debug=False,
                   num_devices=NCORES)

    visT_d = nc.dram_tensor("visT", [NG, VD, RW], F32, kind="ExternalInput")
    nat_d = nc.dram_tensor("visnat", [BC, R, VD], F32, kind="ExternalInput")
    WvT_d = nc.dram_tensor("WvT", [VD, H], F32, kind="ExternalInput")
    WtT_d = nc.dram_tensor("WtT", [TD, H], F32, kind="ExternalInput")
    textT_d = nc.dram_tensor("textT", [TD, BC], F32, kind="ExternalInput")
    bias_d = nc.dram_tensor("biasvb", [128, NH], F32, kind="ExternalInput")
    waT_d = nc.dram_tensor("waT", [128, NH], F32, kind="ExternalInput")
    att_d = nc.dram_tensor("attended", [BC, VD], F32, kind="ExternalOutput")
    aw_d = nc.dram_tensor("attnw", [BC, R], F32, kind="ExternalOutput")

    with tile.TileContext(nc) as tc, ExitStack() as ctx:
        consts = ctx.enter_context(tc.tile_pool(name="consts", bufs=1))
        wtp = ctx.enter_context(tc.tile_pool(name="wtp", bufs=3))
        visp = ctx.enter_context(tc.tile_pool(name="visp", bufs=24))
        natp = ctx.enter_context(tc.tile_pool(name="natp", bufs=2))
        combp = ctx.enter_context(tc.tile_pool(name="combp", bufs=4))
        smallp = ctx.enter_context(tc.tile_pool(name="smallp", bufs=3))
        dramp = ctx.enter_context(tc.tile_pool(name="dramp", bufs=3, space="DRAM"))
        vtps = ctx.enter_context(tc.tile_pool(name="vtps", bufs=3, space="PSUM"))
        scps = ctx.enter_context(tc.tile_pool(name="scps", bufs=2, space="PSUM"))
        atps = ctx.enter_context(tc.tile_pool(name="atps", bufs=2, space="PSUM"))
        tps = ctx.enter_context(tc.tile_pool(name="tps", bufs=1, space="PSUM"))

        # ---- resident constants ----
        # small constants via gpsimd (SWDGE) so they land immediately;
        # WtT via the scalar HWDGE ring; WvT + visT share the sync ring with
        # g0's vis chunks interleaved so the first matmuls start early.
        textT_sb = consts.tile([128, KT, BC], F32R, tag="textT")
        nc.gpsimd.dma_start(
            textT_sb[:],
            textT_d[:, :].rearrange("(k p) b -> p k b", p=128).bitcast(F32R))
        bias_sb = consts.tile([128, NH], F32, tag="bias")
        nc.gpsimd.dma_start(bias_sb[:], bias_d[:, :])
        waT_sb = consts.tile([128, NH], F32R, tag="waT")
        nc.gpsimd.dma_start(waT_sb[:], waT_d[:, :].bitcast(F32R))
        tT_sb = consts.tile([128, NH, BC], F32, tag="tT")
        wt_re = WtT_d[:, :].rearrange("(k p) (h m) -> p k h m", p=128, m=128)
        wt_chunks = []
        for k in range(KT):
            wt = wtp.tile([128, NH, 128], F32R, tag="wt")
            nc.scalar.dma_start(wt[:], wt_re[:, k, :, :].bitcast(F32R))
            wt_chunks.append(wt)

        WvT_sb = consts.tile([128, KV, NH, 128], F32R, tag="WvT")
        wv_re = WvT_d[:, :].rearrange("(k p) (h m) -> p k h m", p=128, m=128)

        vis_pending = []  # deferred first-rowgroup loads, interleaved with WvT

        def load_vis(g):
            tiles = []
            for k in range(KV):
                vk = visp.tile([128, RW], F32R, tag="vis")
                nc.sync.dma_start(
                    vk[:], visT_d[g, k * 128:(k + 1) * 128, :].bitcast(F32R))
                tiles.append(vk)
            return tiles

        # interleave vis(g0) chunk k with WvT chunk k on the sync ring
        g0_tiles = []
        for k in range(KV):
            vk = visp.tile([128, RW], F32R, tag="vis")
            nc.sync.dma_start(
                vk[:], visT_d[0, k * 128:(k + 1) * 128, :].bitcast(F32R))
            g0_tiles.append(vk)
            nc.sync.dma_start(WvT_sb[:, k, :, :], wv_re[:, k, :, :].bitcast(F32R))

        # ---- phase 0: tT[h, b] = (WtT.T @ textT)[h, b] + (bv + bt)[h] ----
        # k-outer so each WtT chunk dies after 8 matmuls; partials are
        # complete per-matmul groups (start=stop=True) accumulated on DVE.
        for k in range(KT):
            t_ps = tps.tile([128, NH, BC], F32, tag="tps")
            for h in range(NH):
                nc.tensor.matmul(t_ps[:, h, :], wt_chunks[k][:, h, :],
                                 textT_sb[:, k, :], start=True, stop=True)
            if k == 0:
                nc.vector.tensor_copy(tT_sb[:], t_ps[:])
            else:
                nc.vector.tensor_add(tT_sb[:], tT_sb[:], t_ps[:])
        for h in range(NH):
            nc.vector.tensor_scalar_add(tT_sb[:, h, :], tT_sb[:, h, :],
                                        bias_sb[:, h:h + 1])

        # ---- phase 1 ----
        def emit_head(g, vis_k):
            sc_ps = scps.tile([1, RW], F32, tag="sc")
            for h in range(NH):
                vt = vtps.tile([128, RW], F32, tag="vt")
                for k in range(KV):
                    nc.tensor.matmul(vt[:], WvT_sb[:, k, h, :], vis_k[k][:],
                                     start=(k == 0), stop=(k == KV - 1))
                comb = combp.tile([128, RW], F32R, tag="comb")
                for j in range(RGB):
                    b = g * RGB + j
                    nc.scalar.activation(
                        comb[:, j * R:(j + 1) * R], vt[:, j * R:(j + 1) * R],
                        AF.Tanh, bias=tT_sb[:, h, b:b + 1], scale=1.0)
                nc.tensor.matmul(sc_ps[:], waT_sb[:, h:h + 1], comb[:],
                                 start=(h == 0), stop=(h == NH - 1))
            return sc_ps

        def emit_softmax(g, sc_ps):
            """Softmax + w scatter + nat prefetch for rowgroup g.
            DVE/ACT/DMA only — no PE instructions."""
            state = []
            for j in range(RGB):
                b = g * RGB + j
                sl = slice(j * R, (j + 1) * R)
                nmax = smallp.tile([1, 1], F32, tag="nmax")
                nc.vector.reduce_max(nmax[:], sc_ps[:, sl],
                                     axis=mybir.AxisListType.X, negate=True)
                ex = smallp.tile([1, R], F32, tag="ex")
                esum = smallp.tile([1, 1], F32, tag="esum")
                nc.scalar.activation(ex[:], sc_ps[:, sl], AF.Exp,
                                     bias=nmax[:], scale=1.0, accum_out=esum[:])
                rec = smallp.tile([1, 1], F32, tag="rec")
                nc.vector.reciprocal(rec[:], esum[:])
                wsm = smallp.tile([1, R], F32, tag="wsm")
                nc.vector.tensor_scalar_mul(wsm[:], ex[:], rec[:])
                nc.gpsimd.dma_start(aw_d[b:b + 1, :], wsm[:])
                # bounce through DRAM to get w onto partitions: [1,R]->[128,RK]
                wb = dramp.tile([1, R], F32, tag="wb")
                nc.gpsimd.dma_start(wb[:], wsm[:])
                wcol = smallp.tile([128, RK], F32R, tag="wcol")
                src = wb[:]
                scat = bass.AP(tensor=src.tensor, offset=src.offset,
                               ap=[[1, 128], [128, RK]])
                nc.gpsimd.dma_start(wcol[:], scat.bitcast(F32R))
                nat = natp.tile([128, RK, VD], F32R, tag="nat")
                nc.scalar.dma_start(
                    nat[:],
                    nat_d[b, :, :].rearrange("(rk p) d -> p rk d", p=128).bitcast(F32R))
                state.append((b, wcol, nat))
            return state

        def emit_att(state):
            """PE matvec for the attended outputs (one rowgroup behind)."""
            for b, wcol, nat in state:
                for n in range(ND):
                    att_ps = atps.tile([1, 512], F32, tag="att")
                    for rk in range(RK):
                        nc.tensor.matmul(att_ps[:], wcol[:, rk:rk + 1],
                                         nat[:, rk, n * 512:(n + 1) * 512],
                                         start=(rk == 0), stop=(rk == RK - 1))
                    att_sb = smallp.tile([1, 512], F32, tag="attsb")
                    nc.vector.tensor_copy(att_sb[:], att_ps[:])
                    nc.gpsimd.dma_start(att_d[b:b + 1, n * 512:(n + 1) * 512],
                                        att_sb[:])

        vis_k = g0_tiles
        pending_att = None
        for g in range(NG):
            sc_ps = emit_head(g, vis_k)
            if g + 1 < NG:
                vis_k = load_vis(g + 1)
            sm_state = emit_softmax(g, sc_ps)
            if pending_att is not None:
                emit_att(pending_att)
            pending_att = sm_state
        emit_att(pending_att)

    nc.compile()
    return nc


def _prep_inputs(visual_features, text_features, Wv, bv, Wt, bt, Wa, ba):
    f32 = np.float32
    vis = np.ascontiguousarray(visual_features, dtype=f32)
    txt = np.ascontiguousarray(text_features, dtype=f32)
    WvT = np.ascontiguousarray(np.asarray(Wv, f32).T)                # (VD, H)
    WtT = np.ascontiguousarray(np.asarray(Wt, f32).T)                # (TD, H)
    bias_vb = np.asarray(bv, f32) + np.asarray(bt, f32)              # (H,)
    bias_sb = np.ascontiguousarray(bias_vb.reshape(NH, 128).T)       # (128, NH)
    waT = np.ascontiguousarray(np.asarray(Wa, f32)[0].reshape(NH, 128).T)

    in_maps = []
    for c in range(NCORES):
        vb = vis[c * BC:(c + 1) * BC]                                # (BC, R, VD)
        vT = vb.transpose(0, 2, 1)                                   # (BC, VD, R)
        visT = np.ascontiguousarray(
            vT.reshape(NG, RGB, VD, R).transpose(0, 2, 1, 3).reshape(NG, VD, RW))
        textT = np.ascontiguousarray(txt[c * BC:(c + 1) * BC].T)     # (TD, BC)
        in_maps.append({
            "visT": visT,
            "visnat": np.ascontiguousarray(vb),
            "WvT": WvT,
            "WtT": WtT,
            "textT": textT,
            "biasvb": bias_sb,
            "waT": waT,
        })
    return in_maps


def kernel(visual_features, text_features, Wv, bv, Wt, bt, Wa, ba):
    from concourse.bass_utils import run_bass_kernel_spmd

    trace = bool(os.environ.get("KERNEL_TRACE"))
    if trace:
        try:
            _install_ntff_hook()
        except Exception as e:  # profiling is best-effort
            print("ntff hook install failed:", e)
            trace = False

    if "nc" not in _cache:
        _cache["nc"] = _build_nc()
    nc = _cache["nc"]

    in_maps = _prep_inputs(visual_features, text_features, Wv, bv, Wt, bt,
                           Wa, ba)
    res = run_bass_kernel_spmd(nc, in_maps, list(range(NCORES)), trace=trace)
    if trace and res.exec_time_ns is not None:
        print(f"HW exec time: {res.exec_time_ns} ns")

    attended = np.concatenate([r["attended"] for r in res.results], axis=0)
    attn_w = np.concatenate([r["attnw"] for r in res.results], axis=0)
    return attended.astype(np.float32), attn_w.astype(np.float32)


# revision 9
# speedup vs baseline: 1.2272x; 1.0820x over previous
"""TRN2 Bass kernel for nn_AttentionMechanism (visual-text attention).

  v = visual @ Wv.T + bv          (128, 256, 1024)
  t = text @ Wt.T + bt            (128, 1024)
  combined = tanh(v + t[:, None])
  scores = combined @ Wa[0] + ba  (128, 256)   [ba dropped: softmax shift-inv]
  attention_weights = softmax(scores, axis=1)
  attended = einsum('br,brd->bd', attention_weights, visual)

Data-parallel across batch on 8 NeuronCores (16 batches/core). Per core the
big projection runs on the TensorEngine in float32r (tf32-like, ~1 cyc/row at
N>=256, ~13-bit mantissa) with the hidden dim on PSUM partitions (vT
orientation) so the per-batch text bias folds into the ScalarEngine's
per-partition activation bias and the score reduction is a PE matvec. The
attended weighted-sum also runs on the PE against a second, natural-layout
copy of visual. Softmax + attended for rowgroup g are deferred until after
rowgroup g+1's matmuls are emitted so the PE never stalls on them.
"""

import sys

if "/opt/trn_rl_repo" not in sys.path:
    sys.path.insert(0, "/opt/trn_rl_repo")

import os
from contextlib import ExitStack

import numpy as np

# ---- problem constants (hardcoded per contract) ----
B, R, VD, TD, H = 128, 256, 2048, 1024, 1024
NCORES = 8
BC = B // NCORES          # 16 batches per core
RGB = 2                   # batches per rowgroup
NG = BC // RGB            # 8 rowgroups
RW = RGB * R              # 512 rows per rowgroup
KV = VD // 128            # 16 k-tiles (visual dim)
KT = TD // 128            # 8 k-tiles (text dim)
NH = H // 128             # 8 h-tiles
ND = VD // 512            # 4 n-tiles for attended output
RK = R // 128             # 2 r-tiles per batch

_cache = {}


def _install_ntff_hook():
    """Register the axon NTFF profile hook the agent image's antenv lacks."""
    import types

    import antenv

    if "antenv.axon_hooks" not in sys.modules:
        mod = types.ModuleType("antenv.axon_hooks")
        mod._hook = None
        mod.set_axon_ntff_profile_hook = lambda h: setattr(mod, "_hook", h)
        mod.get_axon_ntff_profile_hook = lambda: mod._hook
        sys.modules["antenv.axon_hooks"] = mod
        antenv.axon_hooks = mod
    sys.path.insert(0, "/root/.axon_site/trn_agent_boot")
    import trn_boot
    hook = trn_boot._ntff_profile_via_ctypes("/opt/axon/libaxon_pjrt.so")
    sys.modules["antenv.axon_hooks"].set_axon_ntff_profile_hook(hook)


def _build_nc():
    import concourse.bass as bass
    import concourse.tile as tile
    from concourse import bacc, mybir

    F32 = mybir.dt.float32
    F32R = mybir.dt.float32r
    AF = mybir.ActivationFunctionType

    nc = bacc.Bacc("TRN2", target_bir_lowering=False, debug=False,
                   num_devices=NCORES)

    BF16 = mybir.dt.bfloat16
    visT_d = nc.dram_tensor("visT", [NG, VD, RW], BF16, kind="ExternalInput")
    nat_d = nc.dram_tensor("visnat", [BC, R, VD], F32, kind="ExternalInput")
    WvT_d = nc.dram_tensor("WvT", [VD, H], BF16, kind="ExternalInput")
    WtT_d = nc.dram_tensor("WtT", [TD, H], BF16, kind="ExternalInput")
    textT_d = nc.dram_tensor("textT", [TD, BC], BF16, kind="ExternalInput")
    bias_d = nc.dram_tensor("biasvb", [128, NH], F32, kind="ExternalInput")
    waT_d = nc.dram_tensor("waT", [128, NH], BF16, kind="ExternalInput")
    att_d = nc.dram_tensor("attended", [BC, VD], F32, kind="ExternalOutput")
    aw_d = nc.dram_tensor("attnw", [BC, R], F32, kind="ExternalOutput")

    with tile.TileContext(nc) as tc, ExitStack() as ctx:
        consts = ctx.enter_context(tc.tile_pool(name="consts", bufs=1))
        wtp = ctx.enter_context(tc.tile_pool(name="wtp", bufs=3))
        visp = ctx.enter_context(tc.tile_pool(name="visp", bufs=32))
        natp = ctx.enter_context(tc.tile_pool(name="natp", bufs=3))
        combp = ctx.enter_context(tc.tile_pool(name="combp", bufs=4))
        smallp = ctx.enter_context(tc.tile_pool(name="smallp", bufs=3))
        dramp = ctx.enter_context(tc.tile_pool(name="dramp", bufs=3, space="DRAM"))
        vtps = ctx.enter_context(tc.tile_pool(name="vtps", bufs=3, space="PSUM"))
        scps = ctx.enter_context(tc.tile_pool(name="scps", bufs=2, space="PSUM"))
        atps = ctx.enter_context(tc.tile_pool(name="atps", bufs=2, space="PSUM"))
        tps = ctx.enter_context(tc.tile_pool(name="tps", bufs=1, space="PSUM"))

        # ---- resident constants ----
        # small constants via gpsimd (SWDGE) so they land immediately;
        # WtT via the scalar HWDGE ring; WvT + visT share the sync ring with
        # g0's vis chunks interleaved so the first matmuls start early.
        textT_sb = consts.tile([128, KT, BC], BF16, tag="textT")
        nc.gpsimd.dma_start(
            textT_sb[:],
            textT_d[:, :].rearrange("(k p) b -> p k b", p=128))
        bias_sb = consts.tile([128, NH], F32, tag="bias")
        nc.gpsimd.dma_start(bias_sb[:], bias_d[:, :])
        waT_sb = consts.tile([128, NH], BF16, tag="waT")
        nc.gpsimd.dma_start(waT_sb[:], waT_d[:, :])
        tT_sb = consts.tile([128, NH, BC], F32, tag="tT")
        wt_re = WtT_d[:, :].rearrange("(k p) (h m) -> p k h m", p=128, m=128)
        wt_chunks = []
        for k in range(KT):
            wt = wtp.tile([128, NH, 128], BF16, tag="wt")
            nc.scalar.dma_start(wt[:], wt_re[:, k, :, :])
            wt_chunks.append(wt)

        WvT_sb = consts.tile([128, KV, NH, 128], BF16, tag="WvT")
        wv_re = WvT_d[:, :].rearrange("(k p) (h m) -> p k h m", p=128, m=128)

        vis_pending = []  # deferred first-rowgroup loads, interleaved with WvT

        def load_vis(g):
            tiles = []
            for k in range(KV):
                vk = visp.tile([128, RW], BF16, tag="vis")
                nc.sync.dma_start(
                    vk[:], visT_d[g, k * 128:(k + 1) * 128, :])
                tiles.append(vk)
            return tiles

        # interleave vis(g0) chunk k with WvT chunk k on the sync ring
        g0_tiles = []
        for k in range(KV):
            vk = visp.tile([128, RW], BF16, tag="vis")
            nc.sync.dma_start(
                vk[:], visT_d[0, k * 128:(k + 1) * 128, :])
            g0_tiles.append(vk)
            nc.sync.dma_start(WvT_sb[:, k, :, :], wv_re[:, k, :, :])

        # ---- phase 0: tT[h, b] = (WtT.T @ textT)[h, b] + (bv + bt)[h] ----
        # k-outer so each WtT chunk dies after 8 matmuls; partials are
        # complete per-matmul groups (start=stop=True) accumulated on DVE.
        for k in range(KT):
            t_ps = tps.tile([128, NH, BC], F32, tag="tps")
            for h in range(NH):
                nc.tensor.matmul(t_ps[:, h, :], wt_chunks[k][:, h, :],
                                 textT_sb[:, k, :], start=True, stop=True)
            if k == 0:
                nc.vector.tensor_copy(tT_sb[:], t_ps[:])
            else:
                nc.vector.tensor_add(tT_sb[:], tT_sb[:], t_ps[:])
        for h in range(NH):
            nc.vector.tensor_scalar_add(tT_sb[:, h, :], tT_sb[:, h, :],
                                        bias_sb[:, h:h + 1])

        # ---- phase 1 ----
        def emit_head(g, vis_k):
            sc_ps = scps.tile([1, RW], F32, tag="sc")
            for h in range(NH):
                vt = vtps.tile([128, RW], F32, tag="vt")
                for k in range(KV):
                    nc.tensor.matmul(vt[:], WvT_sb[:, k, h, :], vis_k[k][:],
                                     start=(k == 0), stop=(k == KV - 1))
                comb = combp.tile([128, RW], BF16, tag="comb")
                for j in range(RGB):
                    b = g * RGB + j
                    nc.scalar.activation(
                        comb[:, j * R:(j + 1) * R], vt[:, j * R:(j + 1) * R],
                        AF.Tanh, bias=tT_sb[:, h, b:b + 1], scale=1.0)
                nc.tensor.matmul(sc_ps[:], waT_sb[:, h:h + 1], comb[:],
                                 start=(h == 0), stop=(h == NH - 1))
            return sc_ps

        def emit_softmax(g, sc_ps):
            """Softmax + w scatter + nat prefetch for rowgroup g.
            DVE/ACT/DMA only — no PE instructions."""
            state = []
            for j in range(RGB):
                b = g * RGB + j
                sl = slice(j * R, (j + 1) * R)
                nmax = smallp.tile([1, 1], F32, tag="nmax")
                nc.vector.reduce_max(nmax[:], sc_ps[:, sl],
                                     axis=mybir.AxisListType.X, negate=True)
                ex = smallp.tile([1, R], F32, tag="ex")
                esum = smallp.tile([1, 1], F32, tag="esum")
                nc.scalar.activation(ex[:], sc_ps[:, sl], AF.Exp,
                                     bias=nmax[:], scale=1.0, accum_out=esum[:])
                rec = smallp.tile([1, 1], F32, tag="rec")
                nc.vector.reciprocal(rec[:], esum[:])
                wsm = smallp.tile([1, R], F32, tag="wsm")
                nc.vector.tensor_scalar_mul(wsm[:], ex[:], rec[:])
                nc.sync.dma_start(aw_d[b:b + 1, :], wsm[:])
                # bounce through DRAM to get w onto partitions: [1,R]->[128,RK]
                wb = dramp.tile([1, R], F32, tag="wb")
                nc.sync.dma_start(wb[:], wsm[:])
                wcol = smallp.tile([128, RK], F32R, tag="wcol")
                src = wb[:]
                scat = bass.AP(tensor=src.tensor, offset=src.offset,
                               ap=[[1, 128], [128, RK]])
                nc.sync.dma_start(wcol[:], scat.bitcast(F32R))
                nat = natp.tile([128, RK, VD], F32R, tag="nat")
                nc.scalar.dma_start(
                    nat[:],
                    nat_d[b, :, :].rearrange("(rk p) d -> p rk d", p=128).bitcast(F32R))
                state.append((b, wcol, nat))
            return state

        def emit_att(state):
            """PE matvec for the attended outputs (one rowgroup behind)."""
            for b, wcol, nat in state:
                for n in range(ND):
                    att_ps = atps.tile([1, 512], F32, tag="att")
                    for rk in range(RK):
                        nc.tensor.matmul(att_ps[:], wcol[:, rk:rk + 1],
                                         nat[:, rk, n * 512:(n + 1) * 512],
                                         start=(rk == 0), stop=(rk == RK - 1))
                    att_sb = smallp.tile([1, 512], F32, tag="attsb")
                    nc.vector.tensor_copy(att_sb[:], att_ps[:])
                    nc.gpsimd.dma_start(att_d[b:b + 1, n * 512:(n + 1) * 512],
                                        att_sb[:])

        vis_k = g0_tiles
        pending_att = None
        for g in range(NG):
            sc_ps = emit_head(g, vis_k)
            if g + 1 < NG:
                vis_k = load_vis(g + 1)
            sm_state = emit_softmax(g, sc_ps)
            if pending_att is not None:
                emit_att(pending_att)
            pending_att = sm_state
        emit_att(pending_att)

    nc.compile()
    return nc


def _prep_inputs(visual_features, text_features, Wv, bv, Wt, bt, Wa, ba):
    import ml_dtypes
    bf16 = ml_dtypes.bfloat16
    f32 = np.float32
    vis = np.ascontiguousarray(visual_features, dtype=f32)
    txt = np.ascontiguousarray(text_features, dtype=f32)
    WvT = np.ascontiguousarray(np.asarray(Wv, f32).T.astype(bf16))   # (VD, H)
    WtT = np.ascontiguousarray(np.asarray(Wt, f32).T.astype(bf16))   # (TD, H)
    bias_vb = np.asarray(bv, f32) + np.asarray(bt, f32)              # (H,)
    bias_sb = np.ascontiguousarray(bias_vb.reshape(NH, 128).T)       # (128, NH)
    waT = np.ascontiguousarray(np.asarray(Wa, f32)[0].reshape(NH, 128).T.astype(bf16))

    in_maps = []
    for c in range(NCORES):
        vb = vis[c * BC:(c + 1) * BC]                                # (BC, R, VD)
        vT = vb.transpose(0, 2, 1)                                   # (BC, VD, R)
        visT = np.ascontiguousarray(
            vT.reshape(NG, RGB, VD, R).transpose(0, 2, 1, 3)
            .reshape(NG, VD, RW).astype(bf16))
        textT = np.ascontiguousarray(txt[c * BC:(c + 1) * BC].T.astype(bf16))
        in_maps.append({
            "visT": visT,
            "visnat": np.ascontiguousarray(vb),
            "WvT": WvT,
            "WtT": WtT,
            "textT": textT,
            "biasvb": bias_sb,
            "waT": waT,
        })
    return in_maps


def kernel(visual_features, text_features, Wv, bv, Wt, bt, Wa, ba):
    from concourse.bass_utils import run_bass_kernel_spmd

    trace = bool(os.environ.get("KERNEL_TRACE"))
    if trace:
        try:
            _install_ntff_hook()
        except Exception as e:  # profiling is best-effort
            print("ntff hook install failed:", e)
            trace = False

    if "nc" not in _cache:
        _cache["nc"] = _build_nc()
    nc = _cache["nc"]

    in_maps = _prep_inputs(visual_features, text_features, Wv, bv, Wt, bt,
                           Wa, ba)
    res = run_bass_kernel_spmd(nc, in_maps, list(range(NCORES)), trace=trace)
    if trace and res.exec_time_ns is not None:
        print(f"HW exec time: {res.exec_time_ns} ns")

    attended = np.concatenate([r["attended"] for r in res.results], axis=0)
    attn_w = np.concatenate([r["attnw"] for r in res.results], axis=0)
    return attended.astype(np.float32), attn_w.astype(np.float32)


# revision 11
# speedup vs baseline: 1.6217x; 1.3215x over previous
"""TRN2 Bass kernel for nn_AttentionMechanism (visual-text attention).

  v = visual @ Wv.T + bv          (128, 256, 1024)
  t = text @ Wt.T + bt            (128, 1024)
  combined = tanh(v + t[:, None])
  scores = combined @ Wa[0] + ba  (128, 256)   [ba dropped: softmax shift-inv]
  attention_weights = softmax(scores, axis=1)
  attended = einsum('br,brd->bd', attention_weights, visual)

Data-parallel across batch on 8 NeuronCores (16 batches/core). Per core the
big projection runs on the TensorEngine in bf16 with the hidden dim on PSUM
partitions (vT orientation) so the per-batch text bias folds into the
ScalarEngine's per-partition activation bias. The score matvec uses Wa
replicated across all 128 PE columns, so scores (and then softmax) come out
replicated on every partition — the attention weights are directly usable as
a free-dim vector by every DVE lane, and the attended weighted-sum runs as
DVE mul+reduce against the already-resident transposed visual tiles (visual
is read from HBM exactly once).
"""

import sys

if "/opt/trn_rl_repo" not in sys.path:
    sys.path.insert(0, "/opt/trn_rl_repo")

import os
from contextlib import ExitStack

import numpy as np

# ---- problem constants (hardcoded per contract) ----
B, R, VD, TD, H = 128, 256, 2048, 1024, 1024
NCORES = 8
BC = B // NCORES          # 16 batches per core
RGB = 2                   # batches per rowgroup
NG = BC // RGB            # 8 rowgroups
RW = RGB * R              # 512 rows per rowgroup
KV = VD // 128            # 16 k-tiles (visual dim)
KT = TD // 128            # 8 k-tiles (text dim)
NH = H // 128             # 8 h-tiles

_cache = {}


def _install_ntff_hook():
    """Register the axon NTFF profile hook the agent image's antenv lacks."""
    import types

    import antenv

    if "antenv.axon_hooks" not in sys.modules:
        mod = types.ModuleType("antenv.axon_hooks")
        mod._hook = None
        mod.set_axon_ntff_profile_hook = lambda h: setattr(mod, "_hook", h)
        mod.get_axon_ntff_profile_hook = lambda: mod._hook
        sys.modules["antenv.axon_hooks"] = mod
        antenv.axon_hooks = mod
    sys.path.insert(0, "/root/.axon_site/trn_agent_boot")
    import trn_boot
    hook = trn_boot._ntff_profile_via_ctypes("/opt/axon/libaxon_pjrt.so")
    sys.modules["antenv.axon_hooks"].set_axon_ntff_profile_hook(hook)


def _build_nc():
    import concourse.bass as bass  # noqa: F401
    import concourse.tile as tile
    from concourse import bacc, mybir

    F32 = mybir.dt.float32
    BF16 = mybir.dt.bfloat16
    AF = mybir.ActivationFunctionType
    ALU = mybir.AluOpType  # noqa: F841

    nc = bacc.Bacc("TRN2", target_bir_lowering=False, debug=False,
                   num_devices=NCORES)

    visT_d = nc.dram_tensor("visT", [NG, VD, RW], BF16, kind="ExternalInput")
    WvT_d = nc.dram_tensor("WvT", [VD, H], BF16, kind="ExternalInput")
    WtT_d = nc.dram_tensor("WtT", [TD, H], BF16, kind="ExternalInput")
    textT_d = nc.dram_tensor("textT", [TD, BC], BF16, kind="ExternalInput")
    bias_d = nc.dram_tensor("biasvb", [128, NH], F32, kind="ExternalInput")
    warep_d = nc.dram_tensor("warep", [128, NH, 128], BF16, kind="ExternalInput")
    attP_d = nc.dram_tensor("attendedP", [BC, 128, KV], F32, kind="ExternalOutput")
    aw_d = nc.dram_tensor("attnw", [BC, R], F32, kind="ExternalOutput")

    with tile.TileContext(nc) as tc, ExitStack() as ctx:
        consts = ctx.enter_context(tc.tile_pool(name="consts", bufs=1))
        wtp = ctx.enter_context(tc.tile_pool(name="wtp", bufs=3))
        visp = ctx.enter_context(tc.tile_pool(name="visp", bufs=44))
        combp = ctx.enter_context(tc.tile_pool(name="combp", bufs=4))
        smallp = ctx.enter_context(tc.tile_pool(name="smallp", bufs=3))
        tmpp = ctx.enter_context(tc.tile_pool(name="tmpp", bufs=4))
        vtps = ctx.enter_context(tc.tile_pool(name="vtps", bufs=4, space="PSUM"))
        scps = ctx.enter_context(tc.tile_pool(name="scps", bufs=2, space="PSUM"))
        tps = ctx.enter_context(tc.tile_pool(name="tps", bufs=1, space="PSUM"))

        # ---- resident constants ----
        textT_sb = consts.tile([128, KT, BC], BF16, tag="textT")
        nc.gpsimd.dma_start(
            textT_sb[:], textT_d[:, :].rearrange("(k p) b -> p k b", p=128))
        bias_sb = consts.tile([128, NH], F32, tag="bias")
        nc.gpsimd.dma_start(bias_sb[:], bias_d[:, :])
        warep_sb = consts.tile([128, NH, 128], BF16, tag="warep")
        nc.gpsimd.dma_start(warep_sb[:], warep_d[:, :, :])
        tT_sb = consts.tile([128, NH, BC], F32, tag="tT")
        wt_re = WtT_d[:, :].rearrange("(k p) (h m) -> p k h m", p=128, m=128)
        wt_chunks = []
        for k in range(KT):
            wt = wtp.tile([128, NH, 128], BF16, tag="wt")
            nc.scalar.dma_start(wt[:], wt_re[:, k, :, :])
            wt_chunks.append(wt)

        WvT_sb = consts.tile([128, KV, NH, 128], BF16, tag="WvT")
        wv_re = WvT_d[:, :].rearrange("(k p) (h m) -> p k h m", p=128, m=128)

        def load_vis(g, interleave_wv=False):
            tiles = []
            for k in range(KV):
                vk = visp.tile([128, RW], BF16, tag="vis")
                nc.sync.dma_start(vk[:], visT_d[g, k * 128:(k + 1) * 128, :])
                tiles.append(vk)
                if interleave_wv:
                    nc.sync.dma_start(WvT_sb[:, k, :, :], wv_re[:, k, :, :])
            return tiles

        g0_tiles = load_vis(0, interleave_wv=True)

        # ---- phase 0: tT[h, b] = (WtT.T @ textT)[h, b] + (bv + bt)[h] ----
        # k-outer so each WtT chunk dies after 8 matmuls; partials are
        # complete per-matmul groups (start=stop=True) accumulated on DVE.
        for k in range(KT):
            t_ps = tps.tile([128, NH, BC], F32, tag="tps")
            for h in range(NH):
                nc.tensor.matmul(t_ps[:, h, :], wt_chunks[k][:, h, :],
                                 textT_sb[:, k, :], start=True, stop=True)
            if k == 0:
                nc.vector.tensor_copy(tT_sb[:], t_ps[:])
            else:
                nc.vector.tensor_add(tT_sb[:], tT_sb[:], t_ps[:])
        for h in range(NH):
            nc.vector.tensor_scalar_add(tT_sb[:, h, :], tT_sb[:, h, :],
                                        bias_sb[:, h:h + 1])

        # ---- phase 1 ----
        def emit_head(g, vis_k):
            sc_ps = scps.tile([128, RW], F32, tag="sc")
            for h in range(NH):
                vt = vtps.tile([128, RW], F32, tag="vt")
                for k in range(KV):
                    nc.tensor.matmul(vt[:], WvT_sb[:, k, h, :], vis_k[k][:],
                                     start=(k == 0), stop=(k == KV - 1))
                comb = combp.tile([128, RW], BF16, tag="comb")
                for j in range(RGB):
                    b = g * RGB + j
                    nc.scalar.activation(
                        comb[:, j * R:(j + 1) * R], vt[:, j * R:(j + 1) * R],
                        AF.Tanh, bias=tT_sb[:, h, b:b + 1], scale=1.0)
                # Wa replicated to all 128 columns -> scores on every partition
                nc.tensor.matmul(sc_ps[:], warep_sb[:, h, :], comb[:],
                                 start=(h == 0), stop=(h == NH - 1))
            return sc_ps

        def emit_tail(g, sc_ps, vis_k):
            for j in range(RGB):
                b = g * RGB + j
                sl = slice(j * R, (j + 1) * R)
                nmax = smallp.tile([128, 1], F32, tag="nmax")
                nc.vector.reduce_max(nmax[:], sc_ps[:, sl],
                                     axis=mybir.AxisListType.X, negate=True)
                ex = smallp.tile([128, R], F32, tag="ex")
                esum = smallp.tile([128, 1], F32, tag="esum")
                nc.scalar.activation(ex[:], sc_ps[:, sl], AF.Exp,
                                     bias=nmax[:], scale=1.0, accum_out=esum[:])
                rec = smallp.tile([128, 1], F32, tag="rec")
                nc.vector.reciprocal(rec[:], esum[:])
                wsm = smallp.tile([128, R], F32, tag="wsm")
                nc.vector.tensor_scalar_mul(wsm[:], ex[:], rec[:])
                nc.gpsimd.dma_start(aw_d[b:b + 1, :], wsm[0:1, :])
                # attended: DVE mul+reduce over the resident visT tiles
                attcol = smallp.tile([128, KV], F32, tag="attcol")
                for k in range(KV):
                    tmp = tmpp.tile([128, R], F32, tag="tmp")
                    nc.vector.tensor_mul(tmp[:], vis_k[k][:, sl], wsm[:])
                    nc.vector.reduce_sum(attcol[:, k:k + 1], tmp[:],
                                         axis=mybir.AxisListType.X)
                nc.gpsimd.dma_start(attP_d[b, :, :], attcol[:])

        vis_k = g0_tiles
        for g in range(NG):
            sc_ps = emit_head(g, vis_k)
            nxt = load_vis(g + 1) if g + 1 < NG else None
            emit_tail(g, sc_ps, vis_k)
            vis_k = nxt

    nc.compile()
    return nc


def _prep_inputs(visual_features, text_features, Wv, bv, Wt, bt, Wa, ba):
    import ml_dtypes
    bf16 = ml_dtypes.bfloat16
    f32 = np.float32
    vis = np.ascontiguousarray(visual_features, dtype=f32)
    txt = np.ascontiguousarray(text_features, dtype=f32)
    WvT = np.ascontiguousarray(np.asarray(Wv, f32).T.astype(bf16))   # (VD, H)
    WtT = np.ascontiguousarray(np.asarray(Wt, f32).T.astype(bf16))   # (TD, H)
    bias_vb = np.asarray(bv, f32) + np.asarray(bt, f32)              # (H,)
    bias_sb = np.ascontiguousarray(bias_vb.reshape(NH, 128).T)       # (128, NH)
    waT = np.asarray(Wa, f32)[0].reshape(NH, 128).T.astype(bf16)     # (128, NH)
    warep = np.ascontiguousarray(
        np.repeat(waT[:, :, None], 128, axis=2))                     # (128, NH, 128)

    in_maps = []
    for c in range(NCORES):
        vb = vis[c * BC:(c + 1) * BC]                                # (BC, R, VD)
        vT = vb.transpose(0, 2, 1)                                   # (BC, VD, R)
        visT = np.ascontiguousarray(
            vT.reshape(NG, RGB, VD, R).transpose(0, 2, 1, 3)
            .reshape(NG, VD, RW).astype(bf16))
        textT = np.ascontiguousarray(txt[c * BC:(c + 1) * BC].T.astype(bf16))
        in_maps.append({
            "visT": visT,
            "WvT": WvT,
            "WtT": WtT,
            "textT": textT,
            "biasvb": bias_sb,
            "warep": warep,
        })
    return in_maps


def kernel(visual_features, text_features, Wv, bv, Wt, bt, Wa, ba):
    from concourse.bass_utils import run_bass_kernel_spmd

    trace = bool(os.environ.get("KERNEL_TRACE"))
    if trace:
        try:
            _install_ntff_hook()
        except Exception as e:  # profiling is best-effort
            print("ntff hook install failed:", e)
            trace = False

    if "nc" not in _cache:
        _cache["nc"] = _build_nc()
    nc = _cache["nc"]

    in_maps = _prep_inputs(visual_features, text_features, Wv, bv, Wt, bt,
                           Wa, ba)
    res = run_bass_kernel_spmd(nc, in_maps, list(range(NCORES)), trace=trace)
    if trace and res.exec_time_ns is not None:
        print(f"HW exec time: {res.exec_time_ns} ns")

    # attendedP[b, p, k] = attended[b, k*128 + p]
    att_parts = [r["attendedP"].transpose(0, 2, 1).reshape(BC, VD)
                 for r in res.results]
    attended = np.concatenate(att_parts, axis=0)
    attn_w = np.concatenate([r["attnw"] for r in res.results], axis=0)
    return attended.astype(np.float32), attn_w.astype(np.float32)


# revision 12
# speedup vs baseline: 1.6463x; 1.0152x over previous
"""TRN2 Bass kernel for nn_AttentionMechanism (visual-text attention).

  v = visual @ Wv.T + bv          (128, 256, 1024)
  t = text @ Wt.T + bt            (128, 1024)
  combined = tanh(v + t[:, None])
  scores = combined @ Wa[0] + ba  (128, 256)   [ba dropped: softmax shift-inv]
  attention_weights = softmax(scores, axis=1)
  attended = einsum('br,brd->bd', attention_weights, visual)

Data-parallel across batch on 8 NeuronCores (16 batches/core). Per core the
big projection runs on the TensorEngine in bf16 with the hidden dim on PSUM
partitions (vT orientation) so the per-batch text bias folds into the
ScalarEngine's per-partition activation bias. The score matvec uses Wa
replicated across all 128 PE columns, so scores (and then softmax) come out
replicated on every partition — the attention weights are directly usable as
a free-dim vector by every DVE lane, and the attended weighted-sum runs as
DVE mul+reduce against the already-resident transposed visual tiles (visual
is read from HBM exactly once).
"""

import sys

if "/opt/trn_rl_repo" not in sys.path:
    sys.path.insert(0, "/opt/trn_rl_repo")

import os
from contextlib import ExitStack

import numpy as np

# ---- problem constants (hardcoded per contract) ----
B, R, VD, TD, H = 128, 256, 2048, 1024, 1024
NCORES = 8
BC = B // NCORES          # 16 batches per core
RGB = 2                   # batches per rowgroup
NG = BC // RGB            # 8 rowgroups
RW = RGB * R              # 512 rows per rowgroup
KV = VD // 128            # 16 k-tiles (visual dim)
KT = TD // 128            # 8 k-tiles (text dim)
NH = H // 128             # 8 h-tiles

_cache = {}


def _install_ntff_hook():
    """Register the axon NTFF profile hook the agent image's antenv lacks."""
    import types

    import antenv

    if "antenv.axon_hooks" not in sys.modules:
        mod = types.ModuleType("antenv.axon_hooks")
        mod._hook = None
        mod.set_axon_ntff_profile_hook = lambda h: setattr(mod, "_hook", h)
        mod.get_axon_ntff_profile_hook = lambda: mod._hook
        sys.modules["antenv.axon_hooks"] = mod
        antenv.axon_hooks = mod
    sys.path.insert(0, "/root/.axon_site/trn_agent_boot")
    import trn_boot
    hook = trn_boot._ntff_profile_via_ctypes("/opt/axon/libaxon_pjrt.so")
    sys.modules["antenv.axon_hooks"].set_axon_ntff_profile_hook(hook)


def _build_nc():
    import concourse.bass as bass  # noqa: F401
    import concourse.tile as tile
    from concourse import bacc, mybir

    F32 = mybir.dt.float32
    BF16 = mybir.dt.bfloat16
    AF = mybir.ActivationFunctionType
    ALU = mybir.AluOpType  # noqa: F841

    nc = bacc.Bacc("TRN2", target_bir_lowering=False, debug=False,
                   num_devices=NCORES)

    visT_d = nc.dram_tensor("visT", [NG, VD, RW], BF16, kind="ExternalInput")
    WvT_d = nc.dram_tensor("WvT", [VD, H], BF16, kind="ExternalInput")
    WtT_d = nc.dram_tensor("WtT", [TD, H], BF16, kind="ExternalInput")
    textT_d = nc.dram_tensor("textT", [TD, BC], BF16, kind="ExternalInput")
    bias_d = nc.dram_tensor("biasvb", [128, NH], F32, kind="ExternalInput")
    warep_d = nc.dram_tensor("warep", [128, NH, 128], BF16, kind="ExternalInput")
    attP_d = nc.dram_tensor("attendedP", [BC, 128, KV], F32, kind="ExternalOutput")
    aw_d = nc.dram_tensor("attnw", [BC, R], F32, kind="ExternalOutput")

    with tile.TileContext(nc) as tc, ExitStack() as ctx:
        consts = ctx.enter_context(tc.tile_pool(name="consts", bufs=1))
        wtp = ctx.enter_context(tc.tile_pool(name="wtp", bufs=3))
        visp = ctx.enter_context(tc.tile_pool(name="visp", bufs=3))
        combp = ctx.enter_context(tc.tile_pool(name="combp", bufs=4))
        smallp = ctx.enter_context(tc.tile_pool(name="smallp", bufs=3))
        tmpp = ctx.enter_context(tc.tile_pool(name="tmpp", bufs=3))
        vtps = ctx.enter_context(tc.tile_pool(name="vtps", bufs=4, space="PSUM"))
        scps = ctx.enter_context(tc.tile_pool(name="scps", bufs=2, space="PSUM"))
        tps = ctx.enter_context(tc.tile_pool(name="tps", bufs=1, space="PSUM"))

        # ---- resident constants ----
        textT_sb = consts.tile([128, KT, BC], BF16, tag="textT")
        nc.gpsimd.dma_start(
            textT_sb[:], textT_d[:, :].rearrange("(k p) b -> p k b", p=128))
        bias_sb = consts.tile([128, NH], F32, tag="bias")
        nc.gpsimd.dma_start(bias_sb[:], bias_d[:, :])
        warep_sb = consts.tile([128, NH, 128], BF16, tag="warep")
        nc.gpsimd.dma_start(warep_sb[:], warep_d[:, :, :])
        tT_sb = consts.tile([128, NH, BC], F32, tag="tT")
        wt_re = WtT_d[:, :].rearrange("(k p) (h m) -> p k h m", p=128, m=128)
        wt_chunks = []
        for k in range(KT):
            wt = wtp.tile([128, NH, 128], BF16, tag="wt")
            nc.scalar.dma_start(wt[:], wt_re[:, k, :, :])
            wt_chunks.append(wt)

        WvT_sb = consts.tile([128, KV, NH, 128], BF16, tag="WvT")
        wv_re = WvT_d[:, :].rearrange("(k p) (h m) -> p k h m", p=128, m=128)

        def load_vis(g, interleave_wv=False):
            vt_tile = visp.tile([128, KV, RW], BF16, tag="vis")
            for k in range(KV):
                nc.sync.dma_start(vt_tile[:, k, :],
                                  visT_d[g, k * 128:(k + 1) * 128, :])
                if interleave_wv:
                    nc.sync.dma_start(WvT_sb[:, k, :, :], wv_re[:, k, :, :])
            return vt_tile

        g0_tiles = load_vis(0, interleave_wv=True)

        # ---- phase 0: tT[h, b] = (WtT.T @ textT)[h, b] + (bv + bt)[h] ----
        # k-outer so each WtT chunk dies after 8 matmuls; partials are
        # complete per-matmul groups (start=stop=True) accumulated on DVE.
        for k in range(KT):
            t_ps = tps.tile([128, NH, BC], F32, tag="tps")
            for h in range(NH):
                nc.tensor.matmul(t_ps[:, h, :], wt_chunks[k][:, h, :],
                                 textT_sb[:, k, :], start=True, stop=True)
            if k == 0:
                nc.vector.tensor_copy(tT_sb[:], t_ps[:])
            else:
                nc.vector.tensor_add(tT_sb[:], tT_sb[:], t_ps[:])
        for h in range(NH):
            nc.vector.tensor_scalar_add(tT_sb[:, h, :], tT_sb[:, h, :],
                                        bias_sb[:, h:h + 1])

        # ---- phase 1 ----
        def emit_head(g, vis_k):
            sc_ps = scps.tile([128, RW], F32, tag="sc")
            for h in range(NH):
                vt = vtps.tile([128, RW], F32, tag="vt")
                for k in range(KV):
                    nc.tensor.matmul(vt[:], WvT_sb[:, k, h, :], vis_k[:, k, :],
                                     start=(k == 0), stop=(k == KV - 1))
                comb = combp.tile([128, RW], BF16, tag="comb")
                for j in range(RGB):
                    b = g * RGB + j
                    nc.scalar.activation(
                        comb[:, j * R:(j + 1) * R], vt[:, j * R:(j + 1) * R],
                        AF.Tanh, bias=tT_sb[:, h, b:b + 1], scale=1.0)
                # Wa replicated to all 128 columns -> scores on every partition
                nc.tensor.matmul(sc_ps[:], warep_sb[:, h, :], comb[:],
                                 start=(h == 0), stop=(h == NH - 1))
            return sc_ps

        def emit_tail(g, sc_ps, vis_k):
            for j in range(RGB):
                b = g * RGB + j
                sl = slice(j * R, (j + 1) * R)
                nmax = smallp.tile([128, 1], F32, tag="nmax")
                nc.vector.reduce_max(nmax[:], sc_ps[:, sl],
                                     axis=mybir.AxisListType.X, negate=True)
                ex = smallp.tile([128, R], F32, tag="ex")
                esum = smallp.tile([128, 1], F32, tag="esum")
                nc.scalar.activation(ex[:], sc_ps[:, sl], AF.Exp,
                                     bias=nmax[:], scale=1.0, accum_out=esum[:])
                rec = smallp.tile([128, 1], F32, tag="rec")
                nc.vector.reciprocal(rec[:], esum[:])
                wsm = smallp.tile([128, R], F32, tag="wsm")
                nc.vector.tensor_scalar_mul(wsm[:], ex[:], rec[:])
                nc.gpsimd.dma_start(aw_d[b:b + 1, :], wsm[0:1, :])
                # attended: DVE mul+reduce over the resident visT tiles,
                # batched 8 k-tiles per op via 3D APs (wsm broadcast over k)
                attcol = smallp.tile([128, KV], F32, tag="attcol")
                wap = wsm[:]
                KC = 8
                for k0 in range(0, KV, KC):
                    tmp = tmpp.tile([128, KC, R], F32, tag="tmp")
                    wbc = bass.AP(tensor=wap.tensor, offset=wap.offset,
                                  ap=[list(wap.ap[0]), [0, KC]] + list(wap.ap[1:]))
                    nc.vector.tensor_mul(tmp[:], vis_k[:, k0:k0 + KC, sl], wbc)
                    nc.vector.reduce_sum(attcol[:, k0:k0 + KC], tmp[:],
                                         axis=mybir.AxisListType.X)
                nc.gpsimd.dma_start(attP_d[b, :, :], attcol[:])

        vis_k = g0_tiles
        for g in range(NG):
            sc_ps = emit_head(g, vis_k)
            nxt = load_vis(g + 1) if g + 1 < NG else None
            emit_tail(g, sc_ps, vis_k)
            vis_k = nxt

    nc.compile()
    return nc


def _prep_inputs(visual_features, text_features, Wv, bv, Wt, bt, Wa, ba):
    import ml_dtypes
    bf16 = ml_dtypes.bfloat16
    f32 = np.float32
    vis = np.ascontiguousarray(visual_features, dtype=f32)
    txt = np.ascontiguousarray(text_features, dtype=f32)
    WvT = np.ascontiguousarray(np.asarray(Wv, f32).T.astype(bf16))   # (VD, H)
    WtT = np.ascontiguousarray(np.asarray(Wt, f32).T.astype(bf16))   # (TD, H)
    bias_vb = np.asarray(bv, f32) + np.asarray(bt, f32)              # (H,)
    bias_sb = np.ascontiguousarray(bias_vb.reshape(NH, 128).T)       # (128, NH)
    waT = np.asarray(Wa, f32)[0].reshape(NH, 128).T.astype(bf16)     # (128, NH)
    warep = np.ascontiguousarray(
        np.repeat(waT[:, :, None], 128, axis=2))                     # (128, NH, 128)

    in_maps = []
    for c in range(NCORES):
        vb = vis[c * BC:(c + 1) * BC]                                # (BC, R, VD)
        vT = vb.transpose(0, 2, 1)                                   # (BC, VD, R)
        visT = np.ascontiguousarray(
            vT.reshape(NG, RGB, VD, R).transpose(0, 2, 1, 3)
            .reshape(NG, VD, RW).astype(bf16))
        textT = np.ascontiguousarray(txt[c * BC:(c + 1) * BC].T.astype(bf16))
        in_maps.append({
            "visT": visT,
            "WvT": WvT,
            "WtT": WtT,
            "textT": textT,
            "biasvb": bias_sb,
            "warep": warep,
        })
    return in_maps


def kernel(visual_features, text_features, Wv, bv, Wt, bt, Wa, ba):
    from concourse.bass_utils import run_bass_kernel_spmd

    trace = bool(os.environ.get("KERNEL_TRACE"))
    if trace:
        try:
            _install_ntff_hook()
        except Exception as e:  # profiling is best-effort
            print("ntff hook install failed:", e)
            trace = False

    if "nc" not in _cache:
        _cache["nc"] = _build_nc()
    nc = _cache["nc"]

    in_maps = _prep_inputs(visual_features, text_features, Wv, bv, Wt, bt,
                           Wa, ba)
    res = run_bass_kernel_spmd(nc, in_maps, list(range(NCORES)), trace=trace)
    if trace and res.exec_time_ns is not None:
        print(f"HW exec time: {res.exec_time_ns} ns")

    # attendedP[b, p, k] = attended[b, k*128 + p]
    att_parts = [r["attendedP"].transpose(0, 2, 1).reshape(BC, VD)
                 for r in res.results]
    attended = np.concatenate(att_parts, axis=0)
    attn_w = np.concatenate([r["attnw"] for r in res.results], axis=0)
    return attended.astype(np.float32), attn_w.astype(np.float32)


# revision 14
# speedup vs baseline: 1.6815x; 1.0213x over previous
"""TRN2 Bass kernel for nn_AttentionMechanism (visual-text attention).

  v = visual @ Wv.T + bv          (128, 256, 1024)
  t = text @ Wt.T + bt            (128, 1024)
  combined = tanh(v + t[:, None])
  scores = combined @ Wa[0] + ba  (128, 256)   [ba dropped: softmax shift-inv]
  attention_weights = softmax(scores, axis=1)
  attended = einsum('br,brd->bd', attention_weights, visual)

Data-parallel across batch on 8 NeuronCores (16 batches/core). Per core the
big projection runs on the TensorEngine in bf16 with the hidden dim on PSUM
partitions (vT orientation) so the per-batch text bias folds into the
ScalarEngine's per-partition activation bias. The score matvec uses Wa
replicated across all 128 PE columns, so scores (and then softmax) come out
replicated on every partition — the attention weights are directly usable as
a free-dim vector by every DVE lane, and the attended weighted-sum runs as
DVE mul+reduce against the already-resident transposed visual tiles (visual
is read from HBM exactly once).
"""

import sys

if "/opt/trn_rl_repo" not in sys.path:
    sys.path.insert(0, "/opt/trn_rl_repo")

import os
from contextlib import ExitStack

import numpy as np

# ---- problem constants (hardcoded per contract) ----
B, R, VD, TD, H = 128, 256, 2048, 1024, 1024
NCORES = 8
BC = B // NCORES          # 16 batches per core
RGB = 2                   # batches per rowgroup
NG = BC // RGB            # 8 rowgroups
RW = RGB * R              # 512 rows per rowgroup
KV = VD // 128            # 16 k-tiles (visual dim)
KT = TD // 128            # 8 k-tiles (text dim)
NH = H // 128             # 8 h-tiles

_cache = {}


def _install_ntff_hook():
    """Register the axon NTFF profile hook the agent image's antenv lacks."""
    import types

    import antenv

    if "antenv.axon_hooks" not in sys.modules:
        mod = types.ModuleType("antenv.axon_hooks")
        mod._hook = None
        mod.set_axon_ntff_profile_hook = lambda h: setattr(mod, "_hook", h)
        mod.get_axon_ntff_profile_hook = lambda: mod._hook
        sys.modules["antenv.axon_hooks"] = mod
        antenv.axon_hooks = mod
    sys.path.insert(0, "/root/.axon_site/trn_agent_boot")
    import trn_boot
    hook = trn_boot._ntff_profile_via_ctypes("/opt/axon/libaxon_pjrt.so")
    sys.modules["antenv.axon_hooks"].set_axon_ntff_profile_hook(hook)


def _build_nc():
    import concourse.bass as bass  # noqa: F401
    import concourse.tile as tile
    from concourse import bacc, mybir

    F32 = mybir.dt.float32
    BF16 = mybir.dt.bfloat16
    AF = mybir.ActivationFunctionType
    ALU = mybir.AluOpType  # noqa: F841

    nc = bacc.Bacc("TRN2", target_bir_lowering=False, debug=False,
                   num_devices=NCORES)

    visT_d = nc.dram_tensor("visT", [NG, VD, RW], BF16, kind="ExternalInput")
    WvT_d = nc.dram_tensor("WvT", [VD, H], BF16, kind="ExternalInput")
    WtT_d = nc.dram_tensor("WtT", [TD, H], BF16, kind="ExternalInput")
    textT_d = nc.dram_tensor("textT", [TD, BC], BF16, kind="ExternalInput")
    bias_d = nc.dram_tensor("biasvb", [128, NH], F32, kind="ExternalInput")
    warep_d = nc.dram_tensor("warep", [128, NH, 128], BF16, kind="ExternalInput")
    attP_d = nc.dram_tensor("attendedP", [BC, 128, KV], F32, kind="ExternalOutput")
    aw_d = nc.dram_tensor("attnw", [BC, R], F32, kind="ExternalOutput")

    with tile.TileContext(nc) as tc, ExitStack() as ctx:
        consts = ctx.enter_context(tc.tile_pool(name="consts", bufs=1))
        wtp = ctx.enter_context(tc.tile_pool(name="wtp", bufs=4))
        visp = ctx.enter_context(tc.tile_pool(name="visp", bufs=4))
        combp = ctx.enter_context(tc.tile_pool(name="combp", bufs=4))
        smallp = ctx.enter_context(tc.tile_pool(name="smallp", bufs=3))
        tmpp = ctx.enter_context(tc.tile_pool(name="tmpp", bufs=3))
        vtps = ctx.enter_context(tc.tile_pool(name="vtps", bufs=4, space="PSUM"))
        scps = ctx.enter_context(tc.tile_pool(name="scps", bufs=2, space="PSUM"))
        tps = ctx.enter_context(tc.tile_pool(name="tps", bufs=1, space="PSUM"))

        # ---- resident constants ----
        textT_sb = consts.tile([128, KT, BC], BF16, tag="textT")
        nc.sync.dma_start(
            textT_sb[:], textT_d[:, :].rearrange("(k p) b -> p k b", p=128))
        bias_sb = consts.tile([128, NH], F32, tag="bias")
        nc.gpsimd.dma_start(bias_sb[:], bias_d[:, :])
        warep_sb = consts.tile([128, NH, 128], BF16, tag="warep")
        nc.gpsimd.dma_start(warep_sb[:], warep_d[:, :, :])
        tT_sb = consts.tile([128, NH, BC], F32, tag="tT")
        wt_re = WtT_d[:, :].rearrange("(k p) (h m) -> p k h m", p=128, m=128)
        wt_chunks = []
        for k in range(KT):
            wt = wtp.tile([128, NH, 128], BF16, tag="wt")
            eng = nc.scalar if k % 2 == 0 else nc.gpsimd
            eng.dma_start(wt[:], wt_re[:, k, :, :])
            wt_chunks.append(wt)

        WvT_sb = consts.tile([128, KV, NH, 128], BF16, tag="WvT")
        wv_re = WvT_d[:, :].rearrange("(k p) (h m) -> p k h m", p=128, m=128)

        def load_vis(g, interleave_wv=False):
            vt_tile = visp.tile([128, KV, RW], BF16, tag="vis")
            for k in range(KV):
                nc.sync.dma_start(vt_tile[:, k, :],
                                  visT_d[g, k * 128:(k + 1) * 128, :])
                if interleave_wv:
                    nc.sync.dma_start(WvT_sb[:, k, :, :], wv_re[:, k, :, :])
            return vt_tile

        g0_tiles = load_vis(0, interleave_wv=True)

        # ---- phase 0: tT[h, b] = (WtT.T @ textT)[h, b] + (bv + bt)[h] ----
        # k-outer so each WtT chunk dies after 8 matmuls; partials are
        # complete per-matmul groups (start=stop=True) accumulated on DVE.
        for k in range(KT):
            t_ps = tps.tile([128, NH, BC], F32, tag="tps")
            for h in range(NH):
                nc.tensor.matmul(t_ps[:, h, :], wt_chunks[k][:, h, :],
                                 textT_sb[:, k, :], start=True, stop=True)
            if k == 0:
                nc.vector.tensor_copy(tT_sb[:], t_ps[:])
            else:
                nc.vector.tensor_add(tT_sb[:], tT_sb[:], t_ps[:])
        for h in range(NH):
            nc.vector.tensor_scalar_add(tT_sb[:, h, :], tT_sb[:, h, :],
                                        bias_sb[:, h:h + 1])

        # ---- phase 1 ----
        def emit_head(g, vis_k):
            sc_ps = scps.tile([128, RW], F32, tag="sc")
            for h in range(NH):
                vt = vtps.tile([128, RW], F32, tag="vt")
                for k in range(KV):
                    nc.tensor.matmul(vt[:], WvT_sb[:, k, h, :], vis_k[:, k, :],
                                     start=(k == 0), stop=(k == KV - 1))
                comb = combp.tile([128, RW], BF16, tag="comb")
                for j in range(RGB):
                    b = g * RGB + j
                    nc.scalar.activation(
                        comb[:, j * R:(j + 1) * R], vt[:, j * R:(j + 1) * R],
                        AF.Tanh, bias=tT_sb[:, h, b:b + 1], scale=1.0)
                # Wa replicated to all 128 columns -> scores on every partition
                nc.tensor.matmul(sc_ps[:], warep_sb[:, h, :], comb[:],
                                 start=(h == 0), stop=(h == NH - 1))
            return sc_ps

        KC = 8
        NC_CHUNKS = KV // KC

        def emit_tail(g, sc_ps, vis_k):
            wsms, attcols = [], []
            for j in range(RGB):
                b = g * RGB + j
                sl = slice(j * R, (j + 1) * R)
                nmax = smallp.tile([128, 1], F32, tag="nmax")
                nc.vector.reduce_max(nmax[:], sc_ps[:, sl],
                                     axis=mybir.AxisListType.X, negate=True)
                ex = smallp.tile([128, R], F32, tag="ex")
                esum = smallp.tile([128, 1], F32, tag="esum")
                nc.scalar.activation(ex[:], sc_ps[:, sl], AF.Exp,
                                     bias=nmax[:], scale=1.0, accum_out=esum[:])
                rec = smallp.tile([128, 1], F32, tag="rec")
                nc.vector.reciprocal(rec[:], esum[:])
                wsm = smallp.tile([128, R], F32, tag="wsm")
                nc.vector.tensor_scalar_mul(wsm[:], ex[:], rec[:])
                nc.gpsimd.dma_start(aw_d[b:b + 1, :], wsm[0:1, :])
                wsms.append(wsm)
                attcols.append(smallp.tile([128, KV], F32, name=f"attcol{j}", tag=f"attcol{j}"))

            def bcast(wsm):
                wap = wsm[:]
                return bass.AP(tensor=wap.tensor, offset=wap.offset,
                               ap=[list(wap.ap[0]), [0, KC]] + list(wap.ap[1:]))

            # batch j=0 muls on DVE, j=1 muls on GpSimd; reduces on DVE,
            # interleaved so the two batches overlap engine-wise
            tmps = {}
            for c in range(NC_CHUNKS):
                k0 = c * KC
                for j, eng in ((0, nc.vector), (1, nc.gpsimd)):
                    sl = slice(j * R, (j + 1) * R)
                    tmp = tmpp.tile([128, KC, R], F32, name=f"tmp{j}", tag=f"tmp{j}")
                    eng.tensor_mul(tmp[:], vis_k[:, k0:k0 + KC, sl],
                                   bcast(wsms[j]))
                    tmps[(j, c)] = tmp
                nc.vector.reduce_sum(attcols[0][:, k0:k0 + KC], tmps[(0, c)][:],
                                     axis=mybir.AxisListType.X)
            for c in range(NC_CHUNKS):
                k0 = c * KC
                nc.vector.reduce_sum(attcols[1][:, k0:k0 + KC], tmps[(1, c)][:],
                                     axis=mybir.AxisListType.X)
            for j in range(RGB):
                nc.gpsimd.dma_start(attP_d[g * RGB + j, :, :], attcols[j][:])

        vis_k = g0_tiles
        for g in range(NG):
            sc_ps = emit_head(g, vis_k)
            nxt = load_vis(g + 1) if g + 1 < NG else None
            emit_tail(g, sc_ps, vis_k)
            vis_k = nxt

    nc.compile()
    return nc


def _prep_inputs(visual_features, text_features, Wv, bv, Wt, bt, Wa, ba):
    import ml_dtypes
    bf16 = ml_dtypes.bfloat16
    f32 = np.float32
    vis = np.ascontiguousarray(visual_features, dtype=f32)
    txt = np.ascontiguousarray(text_features, dtype=f32)
    WvT = np.ascontiguousarray(np.asarray(Wv, f32).T.astype(bf16))   # (VD, H)
    WtT = np.ascontiguousarray(np.asarray(Wt, f32).T.astype(bf16))   # (TD, H)
    bias_vb = np.asarray(bv, f32) + np.asarray(bt, f32)              # (H,)
    bias_sb = np.ascontiguousarray(bias_vb.reshape(NH, 128).T)       # (128, NH)
    waT = np.asarray(Wa, f32)[0].reshape(NH, 128).T.astype(bf16)     # (128, NH)
    warep = np.ascontiguousarray(
        np.repeat(waT[:, :, None], 128, axis=2))                     # (128, NH, 128)

    in_maps = []
    for c in range(NCORES):
        vb = vis[c * BC:(c + 1) * BC]                                # (BC, R, VD)
        vT = vb.transpose(0, 2, 1)                                   # (BC, VD, R)
        visT = np.ascontiguousarray(
            vT.reshape(NG, RGB, VD, R).transpose(0, 2, 1, 3)
            .reshape(NG, VD, RW).astype(bf16))
        textT = np.ascontiguousarray(txt[c * BC:(c + 1) * BC].T.astype(bf16))
        in_maps.append({
            "visT": visT,
            "WvT": WvT,
            "WtT": WtT,
            "textT": textT,
            "biasvb": bias_sb,
            "warep": warep,
        })
    return in_maps


def kernel(visual_features, text_features, Wv, bv, Wt, bt, Wa, ba):
    from concourse.bass_utils import run_bass_kernel_spmd

    trace = bool(os.environ.get("KERNEL_TRACE"))
    if trace:
        try:
            _install_ntff_hook()
        except Exception as e:  # profiling is best-effort
            print("ntff hook install failed:", e)
            trace = False

    if "nc" not in _cache:
        _cache["nc"] = _build_nc()
    nc = _cache["nc"]

    in_maps = _prep_inputs(visual_features, text_features, Wv, bv, Wt, bt,
                           Wa, ba)
    res = run_bass_kernel_spmd(nc, in_maps, list(range(NCORES)), trace=trace)
    if trace and res.exec_time_ns is not None:
        print(f"HW exec time: {res.exec_time_ns} ns")

    # attendedP[b, p, k] = attended[b, k*128 + p]
    att_parts = [r["attendedP"].transpose(0, 2, 1).reshape(BC, VD)
                 for r in res.results]
    attended = np.concatenate(att_parts, axis=0)
    attn_w = np.concatenate([r["attnw"] for r in res.results], axis=0)
    return attended.astype(np.float32), attn_w.astype(np.float32)
